# revision 10
# baseline (speedup 1.0000x reference)
"""GRU (B=64, T=512, DIN=D=512) on 8 Trainium2 NeuronCores.

Strategy
--------
Data-parallel over batch: each core owns BL = 8 batch rows, weights are
replicated (per the sharding hint).  Per core:

1. Projection phase: xg = X @ W_g + b_g for g in {z, r, h} as bf16 GEMMs
   (X and W converted to bf16 on the host) with W stationary and X^T
   streaming, evacuated by ScalarE Identity-with-bias into an SBUF-resident
   bf16 pre-activation buffer xall[p, g, m, t*BL+b].  Chunks 0-1 run as a
   prologue; later chunks interleave into the scan's PE idle windows.

2. Scan phase: state kept transposed, hT [128 = d%128, KT=4 k-tiles, BL=8]:
   - recurrent matmuls are psum[m] += U[k,m].T @ hmT[k] (U stationary bf16
     with fast-weight-load, state streaming, output already transposed)
   - x-projection terms accumulate into PSUM via an identity matmul
     (start=True) so activations read PSUM directly.
   Critical cycle per step:  r-matmuls -> sigmoid(r) -> rhm -> h-matmuls ->
   tanh -> b2 = zc*hh -> h = c1 + b2.  Everything else (z-matmuls,
   sigmoid(z) [as zc = sigmoid(-pre)], zc*hm, c1 = hm - zc*hm, the eye
   matmuls and interleaved projection work) is scheduled into the idle
   windows of that cycle.  Activations and blends run as single full-width
   [128, 32] ops (per-op overhead on ACT/DVE dominates at this size, so
   fewer/wider beats split halves).

Mask input: reference semantics are h_t = z*(m_{t-1}*h_{t-1}) + ...; for the
all-ones mask the multiply is the identity and the fast path skips it; a
general path streams a host-broadcast shifted mask and adds one DVE mul per
step.
"""

import numpy as np
import ml_dtypes
from contextlib import ExitStack

import concourse.bass as bass
import concourse.bacc as bacc
import concourse.mybir as mybir
import concourse.tile as tile
from concourse.tile import add_dep_helper
from concourse.bass_utils import run_bass_kernel_spmd

FP32 = mybir.dt.float32
BF16 = mybir.dt.bfloat16
AF = mybir.ActivationFunctionType
BF16NP = ml_dtypes.bfloat16

B, T, DIN, D = 64, 512, 512, 512
NCORES = 8
BL = B // NCORES            # 8 batch rows per core
KT = DIN // 128             # 4 contraction tiles
MT = D // 128               # 4 output tiles
P = 128


def build_nc(T_=T, masked=False):
    """Build the single-core SPMD program (identical on all 8 cores)."""
    tl = min(64, T_)                     # steps per chunk
    sch = T_ // tl                       # chunks
    pcw = tl * BL                        # chunk width in columns (512)

    nc = bacc.Bacc(None, target_bir_lowering=False, debug=False)

    xT = nc.dram_tensor("xT", [DIN, T_ * BL], BF16, kind="ExternalInput")
    w_lay = {g: nc.dram_tensor(f"W{g}", [P, KT * D], BF16, kind="ExternalInput")
             for g in "zrh"}
    u_lay = {g: nc.dram_tensor(f"U{g}", [P, KT * D], BF16, kind="ExternalInput")
             for g in "zrh"}
    b4 = {g: nc.dram_tensor(f"b{g}", [P, MT], FP32, kind="ExternalInput")
          for g in "zrh"}
    eye_d = nc.dram_tensor("eye", [P, P], BF16, kind="ExternalInput")
    mb = None
    if masked:
        mb = nc.dram_tensor("mb", [T_, P, KT * BL], FP32, kind="ExternalInput")
    hT_out = nc.dram_tensor("hT_out", [D, BL], FP32, kind="ExternalOutput")

    with tile.TileContext(nc) as tc, ExitStack() as ctx:
        upool = ctx.enter_context(tc.tile_pool(name="upool", bufs=1))
        wpool = ctx.enter_context(tc.tile_pool(name="wpool", bufs=1))
        bp = ctx.enter_context(tc.tile_pool(name="bp", bufs=1))
        xap = ctx.enter_context(tc.tile_pool(name="xap", bufs=1))
        xtp = ctx.enter_context(tc.tile_pool(name="xtp", bufs=2 * KT))
        pproj = ctx.enter_context(
            tc.tile_pool(name="pproj", bufs=2, space="PSUM"))
        psc = ctx.enter_context(tc.tile_pool(name="psc", bufs=2, space="PSUM"))
        sm = ctx.enter_context(tc.tile_pool(name="sm", bufs=3))
        mbp = ctx.enter_context(tc.tile_pool(name="mbp", bufs=2))

        # DMA order matters for the prologue: the projection GEMMs need
        # eye/W/b (and the first xT chunk, emitted below) - the U weights are
        # only needed once the scan starts, so they stream in last.
        eye_sb = upool.tile([P, P], BF16, tag="eye", name="eye")
        nc.scalar.dma_start(eye_sb[:], eye_d[:])
        u_sb = {}
        w_sb = {}
        b_sb = {}
        for g in "zrh":
            w_sb[g] = wpool.tile([P, KT * D], BF16, tag=f"w{g}", name=f"w{g}")
            nc.scalar.dma_start(w_sb[g][:], w_lay[g][:])
            b_sb[g] = bp.tile([P, MT], FP32, tag=f"b{g}", name=f"b{g}")
            nc.scalar.dma_start(b_sb[g][:], b4[g][:])

        def emit_u_dmas():
            for g in "zrh":
                u_sb[g] = upool.tile([P, KT * D], BF16, tag=f"u{g}",
                                     name=f"u{g}")
                nc.sync.dma_start(u_sb[g][:], u_lay[g][:])

        # SBUF-resident pre-activations: [p, gate, m-tile, t*BL+b]
        xall = xap.tile([P, 3, KT, T_ * BL], BF16, tag="xall", name="xall")

        gate_i = {"z": 0, "r": 1, "h": 2}
        xt_tiles = {}

        def emit_xt_dmas(c):
            tiles = []
            for kk in range(KT):
                xt = xtp.tile([P, pcw], BF16, tag="xt", name=f"xt{c}_{kk}")
                nc.sync.dma_start(
                    xt[:], xT[kk * P:(kk + 1) * P, c * pcw:(c + 1) * pcw])
                tiles.append(xt)
            xt_tiles[c] = tiles

        proj_ps = {}

        def emit_proj_half(c, g, m, half, anchor=None, anchor_act=None):
            # half 0: k-tiles 0-1 into a fresh psum buffer; half 1: k-tiles
            # 2-3 + the ACT evacuation.  Halving keeps each injected PE
            # burst (~2x213ns) inside a single scan-step idle window.
            if half == 0:
                proj_ps[(c, g, m)] = pproj.tile(
                    [P, pcw], FP32, tag="pp", name=f"pp{c}{g}{m}")
            ps = proj_ps[(c, g, m)]
            for kk in (0, 1) if half == 0 else (2, 3):
                mm = nc.tensor.matmul(
                    ps[:],
                    w_sb[g][:, kk * D + m * P: kk * D + (m + 1) * P],
                    xt_tiles[c][kk][:],
                    start=(kk == 0), stop=(kk == KT - 1))
                if anchor is not None:
                    # pin this projection burst behind its host step's
                    # recurrent matmuls so the scheduler places it in that
                    # step's idle window instead of flooding the scan
                    add_dep_helper(mm.ins, anchor, sync=False,
                                   reason="proj placement anchor")
                    anchor = None
            if half == 0:
                return None
            ev = nc.scalar.activation(
                xall[:, gate_i[g], m, c * pcw:(c + 1) * pcw], ps[:],
                AF.Identity, bias=b_sb[g][:, m:m + 1])
            if anchor_act is not None:
                # behind the host step's activations, else ScalarE's
                # in-order queue can deadlock against the pinned matmuls
                add_dep_helper(ev.ins, anchor_act, sync=False,
                               reason="proj evac placement anchor")
            return ev

        def emit_proj_unit(c, g, m):
            emit_proj_half(c, g, m, 0)
            return emit_proj_half(c, g, m, 1)

        proj_units = [(c, g, m) for c in range(sch)
                      for g in "zrh" for m in range(MT)]
        # prologue: chunk 0 runs dense before the scan; chunk c+1
        # interleaves into scan chunk c for the rest
        n_pro = min(sch, 1)
        for c in range(n_pro):
            emit_xt_dmas(c)
        emit_u_dmas()
        # first ACT instruction is a sigmoid: the sigmoid_and_others table
        # set also contains identity and tanh, so a single ACT_TABLE_LOAD
        # covers the whole kernel (and runs during the weight DMAs instead
        # of on the scan's critical path)
        warm = sm.tile([P, 1], FP32, tag="warm", name="warm")
        nc.vector.memset(warm[:], 0.0)
        warm_act = nc.scalar.activation(warm[:], warm[:], AF.Sigmoid)

        # keep the PE busy >3.4us while the W/xT DMAs stream in, so the HAM
        # clock gate opens and the projection matmuls run at 2.4GHz instead
        # of cold 1.2GHz
        wps = pproj.tile([P, pcw], FP32, tag="pp", name="warmps")
        for _ in range(64):
            nc.tensor.matmul(wps[:, 0:P], eye_sb[:], eye_sb[:],
                             start=True, stop=True)

        prologue_evacs = []
        for c, g, m in [u for u in proj_units if u[0] < n_pro]:
            ev = emit_proj_unit(c, g, m)
            if not prologue_evacs:
                add_dep_helper(ev.ins, warm_act.ins, sync=False,
                               reason="act table preload")
            prologue_evacs.append(ev.ins)
        rest_halves = [(c, g, m, hf) for c, g, m in proj_units
                       if c >= n_pro for hf in (0, 1)]

        # MM emission order: k-halves outer so all k0/k1 matmuls can start
        # as soon as the first part of the state lands.
        ORD_K = ([(kk, m) for kk in (0, 1) for m in range(MT)]
                 + [(kk, m) for kk in (2, 3) for m in range(MT)])

        def gate_mms(psum, g, rhs, xv, order, barrier=None, after=None):
            # identity matmul accumulates the x-projection into PSUM first
            # (start=True); it has no data deps beyond the projection, so PE
            # can issue it while waiting for rhs.
            idmm = nc.tensor.matmul(psum[:], eye_sb[:], xv[:],
                                    start=True, stop=False)
            if barrier:
                # keep the scheduler from dribbling prologue work into the
                # scan: step 0 starts only after the whole prologue
                for e in barrier:
                    add_dep_helper(idmm.ins, e, sync=True,
                                   reason="prologue barrier")
            stop_mm = None
            for i, (kk, m) in enumerate(order):
                mm = nc.tensor.matmul(
                    psum[:, m],
                    u_sb[g][:, kk * D + m * P: kk * D + (m + 1) * P],
                    rhs[:, kk],
                    start=False,
                    stop=(i == len(order) - 1))
                if i == 0 and after is not None:
                    # keep this gate's matmuls from interleaving into the
                    # previous gate's block - the previous gate's PSUM
                    # completion (which gates an activation on the
                    # critical path) must not be pushed out
                    add_dep_helper(mm.ins, after, sync=False,
                                   reason="gate ordering")
                stop_mm = mm
            return idmm, stop_mm

        h_prev = sm.tile([P, KT, BL], BF16, tag="h", name="h0")
        nc.vector.memset(h_prev[:], 0.0)

        for t in range(T_):
            c = t // tl
            ti = t % tl
            if ti == 0:
                if n_pro <= c + 1 < sch:
                    emit_xt_dmas(c + 1)
                if masked:
                    mb_sb = mbp.tile([P, tl, KT * BL], FP32, tag="m",
                                     name=f"mb{c}")
                    nc.sync.dma_start(
                        mb_sb[:],
                        mb[c * tl:(c + 1) * tl].rearrange("t p x -> p t x"))

            if masked:
                hm = sm.tile([P, KT, BL], BF16, tag="hm")
                nc.vector.tensor_mul(
                    hm[:], h_prev[:],
                    mb_sb[:, ti].rearrange("p (k b) -> p k b", k=KT))
            else:
                hm = h_prev

            xv = xall[:, :, :, t * BL:(t + 1) * BL]

            bar = prologue_evacs if t == 0 else None
            # r gate (on the critical cycle)
            ps_r = psc.tile([P, KT, BL], FP32, tag="pr")
            _, r_stop = gate_mms(ps_r, "r", hm, xv[:, 1], ORD_K, barrier=bar)
            r_sb = sm.tile([P, KT, BL], BF16, tag="r")
            nc.scalar.activation(r_sb[:], ps_r[:], AF.Sigmoid)
            rhm = sm.tile([P, KT, BL], BF16, tag="rhm")
            nc.vector.tensor_mul(rhm[:], r_sb[:], hm[:])

            # z gate (off the cycle: runs in the sigmoid/rhm window)
            ps_z = psc.tile([P, KT, BL], FP32, tag="pz")
            _, z_stop = gate_mms(ps_z, "z", hm, xv[:, 0], ORD_K, barrier=bar,
                                 after=r_stop.ins)
            zc = sm.tile([P, KT, BL], BF16, tag="zc")
            nc.scalar.activation(zc[:], ps_z[:], AF.Sigmoid, scale=-1.0)
            # off-critical-path part of the blend: c1 = hm - zc*hm
            zchm = sm.tile([P, KT, BL], BF16, tag="zchm")
            nc.vector.tensor_mul(zchm[:], zc[:], hm[:])
            c1 = sm.tile([P, KT, BL], BF16, tag="c1")
            nc.vector.tensor_sub(c1[:], hm[:], zchm[:])

            # h candidate
            ps_h = psc.tile([P, KT, BL], FP32, tag="ph")
            _, h_stop = gate_mms(ps_h, "h", rhm, xv[:, 2], ORD_K,
                                 barrier=bar, after=z_stop.ins)

            # critical tail, single full-width ops: hh = tanh(ps_h);
            # h = c1 + zc*hh
            hh = sm.tile([P, KT, BL], BF16, tag="hh")
            b2 = sm.tile([P, KT, BL], BF16, tag="b2")
            h_new = sm.tile([P, KT, BL], BF16, tag="h")
            tanh_op = nc.scalar.activation(hh[:], ps_h[:], AF.Tanh)
            nc.vector.tensor_mul(b2[:], zc[:], hh[:])
            nc.vector.tensor_add(h_new[:], c1[:], b2[:])
            h_prev = h_new

            # interleave hidden projection work into this step's idle
            # windows in half-unit (2-matmul) bursts; chunk c+1 during scan
            # chunk c
            if (rest_halves and c + 1 == rest_halves[0][0]
                    and ti % 2 == 0 and (ti // 2) < 24):
                cc, gg, mm_, hf = rest_halves.pop(0)
                emit_proj_half(cc, gg, mm_, hf, anchor=h_stop.ins,
                               anchor_act=tanh_op.ins)

        hout = sm.tile([P, KT, BL], FP32, tag="hout", name="hout")
        nc.vector.tensor_copy(hout[:], h_prev[:])
        nc.sync.dma_start(
            hT_out.rearrange("(kk p) b -> p kk b", p=P), hout[:])

    nc.compile()
    return nc


_NC_CACHE = {}


def _get_nc(masked):
    if masked not in _NC_CACHE:
        _NC_CACHE[masked] = build_nc(T, masked=masked)
    return _NC_CACHE[masked]


def _w_layout(w):
    # [DIN, D] -> [128, KT*D] with lay[p, kk*D + j] = w[kk*128 + p, j]
    return np.ascontiguousarray(
        np.asarray(w, dtype=np.float32).reshape(KT, P, D)
        .transpose(1, 0, 2).reshape(P, KT * D).astype(BF16NP))


def _b_layout(b):
    return np.ascontiguousarray(
        np.asarray(b, dtype=np.float32).reshape(MT, P).T, dtype=np.float32)


def make_in_maps(X, W_z, U_z, b_z, W_r, U_r, b_r, W_h, U_h, b_h, mask,
                 masked):
    X = np.asarray(X, dtype=np.float32)
    shared = {"eye": np.eye(P, dtype=np.float32).astype(BF16NP)}
    for g, w, u, b in (("z", W_z, U_z, b_z), ("r", W_r, U_r, b_r),
                       ("h", W_h, U_h, b_h)):
        shared[f"W{g}"] = _w_layout(w)
        shared[f"U{g}"] = _w_layout(u)
        shared[f"b{g}"] = _b_layout(b)

    in_maps = []
    for c in range(NCORES):
        bsl = slice(c * BL, (c + 1) * BL)
        m = dict(shared)
        m["xT"] = np.ascontiguousarray(
            X[bsl].transpose(2, 1, 0).reshape(DIN, T * BL).astype(BF16NP))
        if masked:
            msh = np.zeros((T, BL), dtype=np.float32)
            msh[1:] = np.asarray(mask)[bsl, :T - 1].T.astype(np.float32)
            m["mb"] = np.ascontiguousarray(
                np.tile(msh[:, None, :], (1, P, KT)))
        in_maps.append(m)
    return in_maps


def kernel(X, W_z, U_z, b_z, W_r, U_r, b_r, W_h, U_h, b_h, mask):
    mask = np.asarray(mask)
    masked = not bool(np.all(mask[:, :T - 1] == 1))
    nc = _get_nc(masked)
    in_maps = make_in_maps(X, W_z, U_z, b_z, W_r, U_r, b_r, W_h, U_h, b_h,
                           mask, masked)
    res = run_bass_kernel_spmd(nc, in_maps, core_ids=list(range(NCORES)))
    out = np.empty((B, D), dtype=np.float32)
    for c in range(NCORES):
        out[c * BL:(c + 1) * BL] = res.results[c]["hT_out"].T
    return out
